# revision 17
# baseline (speedup 1.0000x reference)
"""Multi-head causal attention (RoPE) on 8 Trainium2 NeuronCores.

Sharding: core c in 0..7 handles batch b = c//4 and head group g = c%4
(4 of 16 heads). Fully fused single-pass pipeline per 512-row block:
  QKV projection (+RoPE) straight into SBUF-resident K/V/Q tiles (bf16),
  transposed flash attention on the block's queries (scores^T [t,q] tiles,
  exp without max-subtraction, causal mask via affine_select, PV + ones
  matmul row sums, reciprocal + broadcast-matmul normalization), the Wo
  projection, then an overlapped bf16 ReduceScatter(add) over {0..3} /
  {4..7} that leaves each core 128 rows of final output per block.

All matmuls run in bf16 (1 cycle/row on the PE) with fp32 PSUM
accumulation; diagonal score/PV/sum matmuls are column-trimmed to the
causal live range.
"""

import math
import sys

import numpy as np

for _p in ("/opt/trn_rl_repo",):
    if _p not in sys.path:
        sys.path.insert(0, _p)

import ml_dtypes

import concourse.bass as bass
import concourse.mybir as mybir
import concourse.tile as tile
from concourse import bacc
from concourse.bass_utils import run_bass_kernel_spmd

F32 = mybir.dt.float32
BF16 = mybir.dt.bfloat16
BF16NP = ml_dtypes.bfloat16

S = 2048        # sequence length
D = 2048        # model dim
HD = 128        # head dim
HPC = 4         # heads per core
SBLK = 512      # block of rows processed per pipeline stage
N_CORES = 8
GROUP = 4       # cores per batch (reduce-scatter group)


def build_nc(s=S, d=D, hpc=HPC, debug=False, reps=1):
    """Build the SPMD Bass program (identical on all cores)."""
    j = hpc * HD          # projected features per core
    nch = d // 128        # contraction chunks
    nblk = s // SBLK      # row blocks
    rows_out = s // GROUP
    scale = 1.0 / math.sqrt(HD)

    nc = bacc.Bacc(
        "TRN2",
        target_bir_lowering=False,
        debug=debug,
        enable_asserts=True,
        num_devices=N_CORES,
    )

    xT = nc.dram_tensor("xT", [d, s], BF16, kind="ExternalInput")
    wqT = nc.dram_tensor("wqT", [d, j], BF16, kind="ExternalInput")
    wkT = nc.dram_tensor("wkT", [d, j], BF16, kind="ExternalInput")
    wvT = nc.dram_tensor("wvT", [d, j], BF16, kind="ExternalInput")
    woT = nc.dram_tensor("woT", [j, d], BF16, kind="ExternalInput")
    cosT = nc.dram_tensor("cosT", [HD, s], BF16, kind="ExternalInput")
    sinH = nc.dram_tensor("sinH", [HD, s], BF16, kind="ExternalInput")
    ones128 = nc.dram_tensor("ones128", [128], BF16, kind="ExternalInput")
    y = nc.dram_tensor("y", [rows_out, d], BF16, kind="ExternalOutput")

    with tile.TileContext(nc) as tc:
        with tc.tile_pool(name="dram", bufs=1, space="DRAM") as dram:
            yp = dram.tile([s, d], BF16, tag="yp")
            rso = dram.tile([rows_out, d], BF16, tag="rso")
            for rep in range(reps):
                _emit_rep(nc, tc, rep, yp, rso, y,
                          xT, wqT, wkT, wvT, woT, cosT, sinH, ones128,
                          s, d, hpc, j, nch, nblk, scale)

    nc.compile()
    return nc


def _emit_rep(nc, tc, rep, yp, rso, y,
              xT, wqT, wkT, wvT, woT, cosT, sinH, ones128,
              s, d, hpc, j, nch, nblk, scale):
    R = str(rep)
    with tc.tile_pool(name="wgt" + R, bufs=1) as wgt, \
         tc.tile_pool(name="ksb" + R, bufs=1) as ksb, \
         tc.tile_pool(name="vsb" + R, bufs=1) as vsb, \
         tc.tile_pool(name="xin" + R, bufs=2) as xin, \
         tc.tile_pool(name="qin" + R, bufs=2 * hpc) as qin, \
         tc.tile_pool(name="rope" + R, bufs=4) as rope, \
         tc.tile_pool(name="pexp" + R, bufs=3) as pexp, \
         tc.tile_pool(name="aot" + R, bufs=hpc + 1) as aot, \
         tc.tile_pool(name="yst" + R, bufs=3) as yst, \
         tc.tile_pool(name="rst" + R, bufs=2) as rst, \
         tc.tile_pool(name="rbt" + R, bufs=2) as rbt, \
         tc.tile_pool(name="psP" + R, bufs=3, space="PSUM") as psP, \
         tc.tile_pool(name="psO" + R, bufs=2, space="PSUM") as psO, \
         tc.tile_pool(name="psM" + R, bufs=1, space="PSUM") as psM, \
         tc.tile_pool(name="psY" + R, bufs=2, space="PSUM") as psY:

            # two softmax-denominator rows, alternated across heads so the
            # next head's sums can start before the previous reciprocal reads
            sm2 = psM.tile([33, SBLK], F32, tag="sm2")

            # ---- persistent SBUF tensors ----
            wq_s = wgt.tile([128, nch * j], BF16, tag="wq")
            wk_s = wgt.tile([128, nch * j], BF16, tag="wk")
            wv_s = wgt.tile([128, nch * j], BF16, tag="wv")
            cos_s = wgt.tile([128, s], BF16, tag="cos")
            sin_s = wgt.tile([128, s], BF16, tag="sin")
            ones_col = wgt.tile([128, 1], BF16, tag="ones_col")
            ones_row = wgt.tile([1, 128], BF16, tag="ones_row")
            wo_s = [wgt.tile([128, d], BF16, tag=f"wo{jc}", name=f"wo{jc}")
                    for jc in range(j // 128)]
            k_sb = [ksb.tile([128, s], BF16, tag=f"k{h}", name=f"k{h}")
                    for h in range(hpc)]
            v_sb = [vsb.tile([128, j], BF16, tag=f"v{cc}", name=f"v{cc}")
                    for cc in range(s // 128)]

            # ---- initial loads ----
            # wk + x block 0 in interleaved pieces so the first projection
            # chain can start as soon as the leading chunks land
            x_first = xin.tile([128, nch * SBLK], BF16, tag="xblk")
            npc = max(1, nch // 8)  # chunks per piece
            for p0 in range(0, nch, npc):
                p1 = p0 + npc
                nc.sync.dma_start(
                    out=wk_s[:, p0 * j:p1 * j].rearrange("p (c j) -> p c j", c=npc),
                    in_=wkT[p0 * 128:p1 * 128, :].rearrange("(c p) j -> p c j", p=128),
                )
                nc.sync.dma_start(
                    out=x_first[:, p0 * SBLK:p1 * SBLK].rearrange(
                        "p (c f) -> p c f", c=npc),
                    in_=xT[p0 * 128:p1 * 128, 0:SBLK].rearrange(
                        "(c p) f -> p c f", p=128),
                )
            nc.sync.dma_start(out=cos_s[:], in_=cosT[:, :])
            nc.sync.dma_start(out=sin_s[:], in_=sinH[:, :])
            nc.sync.dma_start(
                out=ones_col[:],
                in_=ones128[:].rearrange("(p o) -> p o", o=1),
            )
            nc.sync.dma_start(
                out=ones_row[:],
                in_=ones128[:].rearrange("(o f) -> o f", o=1),
            )
            for w_s, w_d in ((wv_s, wvT), (wq_s, wqT)):
                nc.sync.dma_start(
                    out=w_s[:].rearrange("p (c j) -> p c j", c=nch),
                    in_=w_d[:, :].rearrange("(c p) j -> p c j", p=128),
                )
            for jc, t in enumerate(wo_s):
                nc.sync.dma_start(
                    out=t[:],
                    in_=woT[jc * 128:(jc + 1) * 128, :],
                )

            def rope_store(ps, dst_ap, ssl):
                """dst = ps*cos + rotate_half(ps)*sin  (bf16 out)."""
                rot = rope.tile([128, SBLK], F32, tag="rot")
                st = rope.tile([128, SBLK], F32, tag="sta")
                nc.vector.tensor_mul(rot[0:64, :], ps[64:128, :], sin_s[0:64, ssl])
                nc.vector.tensor_mul(rot[64:128, :], ps[0:64, :], sin_s[64:128, ssl])
                nc.vector.tensor_mul(st[:], ps[:], cos_s[:, ssl])
                nc.vector.tensor_add(dst_ap, st[:], rot[:])

            x_s = x_first
            for blk in range(nblk):
                ssl = slice(blk * SBLK, (blk + 1) * SBLK)
                nk = (blk + 1) * (SBLK // 128)

                # ---------- phase A: projections + RoPE (SBUF-resident) ----------
                # k j-tiles -> k_sb[jt][:, ssl]
                for jt in range(hpc):
                    ps = psP.tile([128, SBLK], F32, tag="pp")
                    for c in range(nch):
                        nc.tensor.matmul(
                            ps[:],
                            lhsT=wk_s[:, c * j + jt * 128: c * j + (jt + 1) * 128],
                            rhs=x_s[:, c * SBLK:(c + 1) * SBLK],
                            start=(c == 0),
                            stop=(c == nch - 1),
                        )
                    rope_store(ps, k_sb[jt][:, ssl], ssl)
                # x prefetch for next block (overlaps the rest of this block)
                if blk + 1 < nblk:
                    x_next = xin.tile([128, nch * SBLK], BF16, tag="xblk")
                    nssl = slice((blk + 1) * SBLK, (blk + 2) * SBLK)
                    nc.sync.dma_start(
                        out=x_next[:].rearrange("p (c f) -> p c f", c=nch),
                        in_=xT[:, nssl].rearrange("(c p) f -> p c f", p=128),
                    )
                # v s-subtiles -> v_sb[blk*4+su]  ([t, j] layout)
                for su in range(SBLK // 128):
                    ps = psP.tile([128, j], F32, tag="pp")
                    for c in range(nch):
                        nc.tensor.matmul(
                            ps[:],
                            lhsT=x_s[:, c * SBLK + su * 128: c * SBLK + (su + 1) * 128],
                            rhs=wv_s[:, c * j:(c + 1) * j],
                            start=(c == 0),
                            stop=(c == nch - 1),
                        )
                    nc.scalar.copy(v_sb[blk * (SBLK // 128) + su][:], ps[:])
                # q j-tiles -> q_t[jt] (transient)
                q_t = []
                for jt in range(hpc):
                    ps = psP.tile([128, SBLK], F32, tag="pp")
                    for c in range(nch):
                        nc.tensor.matmul(
                            ps[:],
                            lhsT=wq_s[:, c * j + jt * 128: c * j + (jt + 1) * 128],
                            rhs=x_s[:, c * SBLK:(c + 1) * SBLK],
                            start=(c == 0),
                            stop=(c == nch - 1),
                        )
                    qt_ = qin.tile([128, SBLK], BF16, tag="qt")
                    rope_store(ps, qt_[:], ssl)
                    q_t.append(qt_)
                if blk + 1 < nblk:
                    x_s = x_next

                # ---------- phase B: attention for this q block ----------
                diag0 = blk * (SBLK // 128)   # first diagonal chunk index

                def qoff(c):
                    return (c - diag0) * 128 if c >= diag0 else 0

                stream = [(h, c) for h in range(hpc) for c in range(nk)]
                p_t = {}
                ob_t = {}
                sm_t = {}
                aos = [None] * hpc

                def issue_scores(idx):
                    h, c = stream[idx]
                    qo = qoff(c)
                    sc = psP.tile([128, SBLK], F32, tag="pp")
                    nc.tensor.matmul(
                        sc[:, qo:SBLK],
                        lhsT=k_sb[h][:, c * 128:(c + 1) * 128],
                        rhs=q_t[h][:, qo:SBLK],
                        start=True,
                        stop=True,
                    )
                    p = pexp.tile([128, SBLK], BF16, tag="p")
                    nc.scalar.activation(
                        p[:, qo:SBLK], sc[:, qo:SBLK],
                        mybir.ActivationFunctionType.Exp,
                        scale=scale,
                    )
                    if c >= diag0:
                        # zero the triangle above the causal diagonal
                        nc.gpsimd.affine_select(
                            p[:, qo:qo + 128], p[:, qo:qo + 128],
                            pattern=[[1, 128]],
                            compare_op=mybir.AluOpType.is_ge,
                            fill=0.0,
                            base=0,
                            channel_multiplier=-1,
                        )
                    p_t[(h, c)] = p

                def epilogue(h):
                    # softmax normalization: reciprocal row sums broadcast
                    # across partitions on the (idle) Pool engine
                    rs = rst.tile([1, SBLK], BF16, tag="rs")
                    with nc.allow_low_precision(reason="bf16 softmax denominators"):
                        nc.vector.reciprocal(rs[:], sm_t[h])
                    rb = rbt.tile([128, SBLK], BF16, tag="rb")
                    nc.gpsimd.partition_broadcast(rb[:], rs[:])
                    ao = aot.tile([128, SBLK], BF16, tag="ao")
                    nc.vector.tensor_mul(ao[:], ob_t[h][:], rb[:])
                    aos[h] = ao

                issue_scores(0)
                if len(stream) > 1:
                    issue_scores(1)
                for idx, (h, c) in enumerate(stream):
                    if idx + 2 < len(stream):
                        issue_scores(idx + 2)
                    if c == 0:
                        ob_t[h] = psO.tile([128, SBLK], F32, tag="ob", name="ob")
                        r = (h % 2) * 32
                        sm_t[h] = sm2[r:r + 1, :]
                    # deferred epilogue of the previous head hides the
                    # reciprocal latency under this head's matmuls
                    if c == min(2, nk - 1) and h > 0:
                        epilogue(h - 1)
                    qo = qoff(c)
                    p = p_t.pop((h, c))
                    nc.tensor.matmul(
                        ob_t[h][:, qo:SBLK],
                        lhsT=v_sb[c][:, h * 128:(h + 1) * 128],
                        rhs=p[:, qo:SBLK],
                        start=(c == 0),
                        stop=(c == nk - 1),
                    )
                    nc.tensor.matmul(
                        sm_t[h][:, qo:SBLK],
                        lhsT=ones_col[:],
                        rhs=p[:, qo:SBLK],
                        start=(c == 0),
                        stop=(c == nk - 1),
                    )
                epilogue(hpc - 1)

                # ---------- Wo projection for this block ----------
                for su in range(SBLK // 128):
                    ys = yst.tile([128, d], BF16, tag="ys")
                    for es in range(d // 512):
                        yb = psY.tile([128, 512], F32, tag="yb")
                        for jc in range(j // 128):
                            nc.tensor.matmul(
                                yb[:],
                                lhsT=aos[jc][:, su * 128:(su + 1) * 128],
                                rhs=wo_s[jc][:, es * 512:(es + 1) * 512],
                                start=(jc == 0),
                                stop=(jc == j // 128 - 1),
                            )
                        nc.scalar.copy(ys[:, es * 512:(es + 1) * 512], yb[:])
                    r0 = blk * SBLK + su * 128
                    nc.sync.dma_start(out=yp[r0:r0 + 128, :], in_=ys[:])

                # ---------- overlapped ReduceScatter + output ----------
                nc.gpsimd.collective_compute(
                    "ReduceScatter",
                    mybir.AluOpType.add,
                    replica_groups=[[0, 1, 2, 3], [4, 5, 6, 7]],
                    ins=[yp[ssl, :].opt()],
                    outs=[rso[blk * 128:(blk + 1) * 128, :].opt()],
                )
                nc.sync.dma_start(
                    out=y[blk * 128:(blk + 1) * 128, :],
                    in_=rso[blk * 128:(blk + 1) * 128, :],
                )


def make_rope_tables(s, hd):
    inv_freq = 1.0 / (10000.0 ** (np.arange(0, hd, 2, dtype=np.float32) / hd))
    t = np.arange(s, dtype=np.float32)
    freqs = np.outer(t, inv_freq)  # [s, hd/2]
    cos = np.concatenate([np.cos(freqs), np.cos(freqs)], axis=-1)  # [s, hd]
    sin = np.concatenate([np.sin(freqs), np.sin(freqs)], axis=-1)
    cosT = np.ascontiguousarray(cos.T.astype(np.float32))  # [hd, s]
    sinT = sin.T.astype(np.float32)
    sinH = sinT.copy()
    sinH[: hd // 2] = -sinH[: hd // 2]
    return cosT, np.ascontiguousarray(sinH)


def make_in_maps(x, Wq, Wk, Wv, Wo, s=S, d=D, hpc=HPC):
    j = hpc * HD
    cosT, sinH = make_rope_tables(s, HD)
    cosT = cosT.astype(BF16NP)
    sinH = np.ascontiguousarray(sinH).astype(BF16NP)
    xTs = [np.ascontiguousarray(x[b].T).astype(BF16NP) for b in range(x.shape[0])]
    in_maps = []
    for c in range(N_CORES):
        b, g = divmod(c, GROUP)
        hs = slice(g * j, (g + 1) * j)
        in_maps.append({
            "xT": xTs[b],
            "wqT": np.ascontiguousarray(Wq[hs, :].T).astype(BF16NP),
            "wkT": np.ascontiguousarray(Wk[hs, :].T).astype(BF16NP),
            "wvT": np.ascontiguousarray(Wv[hs, :].T).astype(BF16NP),
            "woT": np.ascontiguousarray(Wo[:, hs].T).astype(BF16NP),
            "cosT": cosT,
            "sinH": sinH,
            "ones128": np.ones(128, BF16NP),
        })
    return in_maps


_NC_CACHE = {}


def get_nc():
    if "nc" not in _NC_CACHE:
        _NC_CACHE["nc"] = build_nc()
    return _NC_CACHE["nc"]


def assemble_output(results, x_shape, s=S, d=D):
    B = x_shape[0]
    out = np.empty((B, s, d), dtype=np.float32)
    nblk = s // SBLK
    for c in range(N_CORES):
        b, g = divmod(c, GROUP)
        yc = np.asarray(results[c]["y"]).astype(np.float32)
        for blk in range(nblk):
            r0 = blk * SBLK + g * 128
            out[b, r0:r0 + 128, :] = yc[blk * 128:(blk + 1) * 128]
    return out


def kernel(x, Wq, Wk, Wv, Wo, n_heads, **_):
    x = np.asarray(x, dtype=np.float32)
    assert int(n_heads) == 16 and x.shape == (2, S, D)
    nc = get_nc()
    in_maps = make_in_maps(
        x,
        np.asarray(Wq, np.float32), np.asarray(Wk, np.float32),
        np.asarray(Wv, np.float32), np.asarray(Wo, np.float32),
    )
    res = run_bass_kernel_spmd(nc, in_maps, list(range(N_CORES)))
    return assemble_output(res.results, x.shape)
